# revision 8
# baseline (speedup 1.0000x reference)
"""Trainium2 Bass kernel for nn_MultiHeadAttention (softmax over HEAD axis).

Problem: B=2, T=2048, D=1024, H=16, HD=64.
  Q,K,V = x@W* + b*;  score = QK^T/32 with causal positions set to -1e10
  weight = softmax(score, axis=HEADS)  -> masked (j>i) entries get exactly 1/16
  out = weight@V;  y = out@Wo + bo

Exact identity used: for row i,
  out_h[i] = sum_{j<=i} w_h[i,j] V_h[j] + (1/16) sum_{j>i} V_h[j]
where w is the head-softmax of unmasked scores. We compute softmax weights
only on causal j-blocks, zero the off-causal entries via 0/1 masks, and add
the (1/16)*suffix-sum(V) correction as a host-precomputed matrix (V comes
from launch A's own output, so the correction is consistent to fp16).

Sharding (8 cores, two launches):
  Launch A: QKV projections, 8-way token-sharded.
  Launch B: attention + out-proj. Core c (q = c%4, batch c//4) handles the
    mirrored 2-block chunks A=(2q, 2q+1), B=(14-2q, 15-2q) of 128-row blocks.
    One SPMD program for all cores: slot A runs 8 j-block positions, slot B
    16; real causal counts are (2q+2, 16-2q) and the rest are padding whose
    weights the per-core mask data zeroes.

Launch B engine split (per 128j x 256i unit):
  PE:   16 score matmuls (fp16, K=64, two PE row-groups packed) +
        16 weight@V matmuls accumulated per 4-unit super-block in PSUM
  ACT:  4x exp (PSUM f32 -> SBUF fp16), nothing else (no table reloads)
  DVE:  Z add-tree (fp16 pairwise), reciprocal, mask*recip, half the
        w = P*(1/Z) muls, PSUM->SBUF drain-adds
  Pool: the other half of the w muls
"""

import numpy as np

import concourse.bass as bass
import concourse.tile as tile
from concourse import bacc, mybir
from concourse.bass_utils import run_bass_kernel_spmd

F16 = mybir.dt.float16
F32 = mybir.dt.float32
AF = mybir.ActivationFunctionType

B, T, D, H, HD = 2, 2048, 1024, 16, 64
NC = 8
NBLK = T // 128          # 16
CNT = (8, 16)            # padded j-position counts for slot A / slot B
NPOS = CNT[0] + CNT[1]   # 24

# head slot order per 4-head score group: even (row-group-0) heads first so
# each PSUM bank only ever receives matmuls from one PE row group.
_GRP_HEADS = [[4 * g, 4 * g + 2, 4 * g + 1, 4 * g + 3] for g in range(4)]
# head -> (group, slot)
_HEAD_SLOT = {}
for _g in range(4):
    for _s, _h in enumerate(_GRP_HEADS[_g]):
        _HEAD_SLOT[_h] = (_g, _s)

_cache: dict = {}


# ----------------------------------------------------------------- launch A
def _build_a(reps=1):
    """QKV projections for a 512-token slice (8-way token-sharded)."""
    nc = bacc.Bacc("TRN2", target_bir_lowering=False, debug=False, num_devices=NC)
    xT = nc.dram_tensor("xT", [128, 8, 512], F16, kind="ExternalInput")
    wq = nc.dram_tensor("wq", [128, 8, D], F16, kind="ExternalInput")
    wk = nc.dram_tensor("wk", [128, 8, D], F16, kind="ExternalInput")
    wv = nc.dram_tensor("wv", [128, 8, D], F16, kind="ExternalInput")
    bqT = nc.dram_tensor("bqT", [128, 8], F32, kind="ExternalInput")
    bkT = nc.dram_tensor("bkT", [128, 8], F32, kind="ExternalInput")
    bv_row = nc.dram_tensor("bv_row", [1, D], F16, kind="ExternalInput")
    qT_o = nc.dram_tensor("qT_o", [128, 8, 512], F16, kind="ExternalOutput")
    kT_o = nc.dram_tensor("kT_o", [128, 8, 512], F16, kind="ExternalOutput")
    v_o = nc.dram_tensor("v_o", [128, 4, D], F16, kind="ExternalOutput")

    from contextlib import nullcontext
    with tile.TileContext(nc) as tc:
        with (tc.For_i(0, reps) if reps > 1 else nullcontext()), \
             tc.tile_pool(name="sg", bufs=1) as sg, \
             tc.tile_pool(name="out", bufs=1) as outp, \
             tc.tile_pool(name="ps", bufs=8, space="PSUM") as ps:
            xt = sg.tile([128, 8, 512], F16, tag="xt")
            nc.sync.dma_start(out=xt[:], in_=xT[:])
            wts = {}
            for nm, dram in (("wq", wq), ("wk", wk), ("wv", wv)):
                wt = sg.tile([128, 8, D], F16, tag=nm)
                nc.sync.dma_start(out=wt[:], in_=dram[:])
                wts[nm] = wt
            bq_sb = sg.tile([128, 8], F32, tag="bq")
            nc.sync.dma_start(out=bq_sb[:], in_=bqT[:])
            bk_sb = sg.tile([128, 8], F32, tag="bk")
            nc.sync.dma_start(out=bk_sb[:], in_=bkT[:])
            bv_sb = sg.tile([1, D], F16, tag="bv")
            nc.sync.dma_start(out=bv_sb[:], in_=bv_row[:])
            ones1 = sg.tile([1, 128], F16, tag="ones1")
            nc.vector.memset(ones1[:], 1.0)

            # Q^T, K^T: out[dout_chunk, t] = W[din, dout].T @ xT[din, t]
            for nm, bias_sb, scale, dst in (
                ("wq", bq_sb, 1.0, qT_o),
                ("wk", bk_sb, 1.0 / 32.0, kT_o),
            ):
                res = outp.tile([128, 8, 512], F16, tag=f"r{nm}")
                for m in range(8):
                    acc = ps.tile([128, 512], F32, tag="acc")
                    for k in range(8):
                        nc.tensor.matmul(
                            acc[:],
                            wts[nm][:, k, m * 128:(m + 1) * 128],
                            xt[:, k, :],
                            start=(k == 0), stop=(k == 7),
                        )
                    nc.scalar.activation(
                        out=res[:, m, :], in_=acc[:], func=AF.Identity,
                        bias=bias_sb[:, m:m + 1], scale=scale,
                    )
                nc.sync.dma_start(out=dst[:], in_=res[:])

            # V natural: out[t_chunk, dout] = xT[din, t_chunk].T @ Wv[din, dout]
            rv = outp.tile([128, 4, D], F16, tag="rv")
            for tcn in range(4):
                for nt in range(2):
                    acc = ps.tile([128, 512], F32, tag="acc")
                    for k in range(8):
                        nc.tensor.matmul(
                            acc[:],
                            xt[:, k, tcn * 128:(tcn + 1) * 128],
                            wts["wv"][:, k, nt * 512:(nt + 1) * 512],
                            start=(k == 0), stop=False,
                        )
                    nc.tensor.matmul(
                        acc[:], ones1[:], bv_sb[:, nt * 512:(nt + 1) * 512],
                        start=False, stop=True,
                    )
                    nc.scalar.activation(
                        out=rv[:, tcn, nt * 512:(nt + 1) * 512], in_=acc[:],
                        func=AF.Copy)
            nc.sync.dma_start(out=v_o[:], in_=rv[:])
    nc.compile()
    return nc


# ----------------------------------------------------------------- launch B
def _chunk_blocks(q):
    """Global 128-row block indices of the two chunks handled by quarter q."""
    return (2 * q, 2 * q + 1), (14 - 2 * q, 15 - 2 * q)


def _build_b(reps=1, stages=5, wmode="gps_t8"):
    """Attention + out-projection, one uniform SPMD program for all cores.

    Per-core inputs:
      qT [128, 8, 512] f16 : Q^T, cols = [chunk A 256 | chunk B 256]
      kT [128, 8, 2048] f16 (pre-scaled 1/32), v [128, 16, 1024] f16
      wo [128, 8, 1024] f16, bo_row [1, 1024] f16
      masks [24, 128, 256] f16 : per position 0/1 weight-keep masks
      corr [2, 128, 8, 256] f16 : (1/16)*suffix-sum-of-V correction laid out
          [chunk, d-within-pair, d-pair-chunk, i-col], added to out^T
    Output: y [512, 1024] f32 (rows = [chunk A | chunk B]).

    weight@V accumulates over ALL of a chunk's units in one PSUM tile:
    only the first matmul touching each bank uses start=True (clearing the
    bank's has_written bits); every other matmul uses start=False, which
    overwrites where the bit is unset and accumulates where it is set.
    """
    nc = bacc.Bacc("TRN2", target_bir_lowering=False, debug=False, num_devices=NC)
    qT = nc.dram_tensor("qT", [128, 8, 512], F16, kind="ExternalInput")
    kT = nc.dram_tensor("kT", [128, 8, T], F16, kind="ExternalInput")
    v = nc.dram_tensor("v", [128, 16, D], F16, kind="ExternalInput")
    wo = nc.dram_tensor("wo", [128, 8, D], F16, kind="ExternalInput")
    bo_row = nc.dram_tensor("bo_row", [1, D], F16, kind="ExternalInput")
    masks = nc.dram_tensor("masks", [NPOS, 128, 256], F16, kind="ExternalInput")
    corr = nc.dram_tensor("corr", [2, 128, 8, 256], F16, kind="ExternalInput")
    y_o = nc.dram_tensor("y", [512, D], F32, kind="ExternalOutput")

    from contextlib import nullcontext
    with tile.TileContext(nc) as tc:
        with (tc.For_i(0, reps) if reps > 1 else nullcontext()), \
             tc.tile_pool(name="sg", bufs=1) as sg, \
             tc.tile_pool(name="pt", bufs=2) as ptp, \
             tc.tile_pool(name="wbuf", bufs=5) as wbuf, \
             tc.tile_pool(name="w2buf", bufs=5) as w2buf, \
             tc.tile_pool(name="tr", bufs=2) as trp, \
             tc.tile_pool(name="mk", bufs=24) as mkp:

            # kt/vt split into per-j-block loads so the For_i reload of
            # block jb can start right after its last reader (chunk B's
            # unit jb) instead of serializing at the rep boundary.
            kt = sg.tile([128, 8, T], F16, tag="kt")
            for jb in range(4):
                nc.sync.dma_start(out=kt[:, :, jb * 128:(jb + 1) * 128],
                                  in_=kT[:, :, jb * 128:(jb + 1) * 128])
            mks = []
            for p_ in range(NPOS):
                mk = mkp.tile([128, 256], F16, tag="mk")
                nc.sync.dma_start(out=mk[:], in_=masks[p_, :, :])
                mks.append(mk)
            for jb in range(4, NBLK):
                nc.sync.dma_start(out=kt[:, :, jb * 128:(jb + 1) * 128],
                                  in_=kT[:, :, jb * 128:(jb + 1) * 128])
            vt = sg.tile([128, 16, D], F16, tag="vt")
            for jb in range(NBLK):
                nc.gpsimd.dma_start(out=vt[:, jb, :], in_=v[:, jb, :])
            qt = sg.tile([128, 8, 512], F16, tag="qt")
            for h_ in range(2):
                nc.scalar.dma_start(out=qt[:, :, h_ * 256:(h_ + 1) * 256],
                                    in_=qT[:, :, h_ * 256:(h_ + 1) * 256])
            wot = sg.tile([128, 8, D], F16, tag="wot")
            nc.gpsimd.dma_start(out=wot[:], in_=wo[:])
            bo_sb = sg.tile([1, D], F16, tag="bo")
            nc.sync.dma_start(out=bo_sb[:], in_=bo_row[:])
            ones1 = sg.tile([1, 128], F16, tag="ones1")
            nc.vector.memset(ones1[:], 1.0)
            cks = []
            for ci in range(2):
                ck = sg.tile([128, 8, 256], F16, tag=f"ck{ci}",
                             name=f"ck{ci}")
                nc.scalar.dma_start(out=ck[:], in_=corr[ci, :, :, :])
                cks.append(ck)

            outTs = [sg.tile([128, 8, 256], F16, tag=f"outT{ci}",
                             name=f"outT{ci}") for ci in range(2)]

            with tc.tile_pool(name="score", bufs=2, space="PSUM") as scp:
                with tc.tile_pool(name="ot", bufs=1, space="PSUM") as otp:
                    for ci in range(2):
                        npos = CNT[ci]
                        coff = ci * 256
                        poff = 0 if ci == 0 else CNT[0]   # mask index offset
                        outT = outTs[ci]

                        pend = []
                        for p in range(npos + 1):
                            # ---------- phase 1 for unit p
                            if p < npos:
                                jb = p
                                pt = ptp.tile([128, 16, 256], F16, tag="pt")
                                for g in range(4):
                                    sc = scp.tile([128, 4, 256], F32, tag="sc")
                                    for hh, h in enumerate(_GRP_HEADS[g]):
                                        c, off = h // 2, (h % 2) * 64
                                        nc.tensor.matmul(
                                            sc[:, hh, :],
                                            kt[off:off + 64, c,
                                               jb * 128:(jb + 1) * 128],
                                            qt[off:off + 64, c,
                                               coff:coff + 256],
                                            start=True, stop=True,
                                            tile_position=(off, 0),
                                        )
                                    nc.scalar.activation(
                                        out=pt[:, 4 * g:4 * g + 4, :],
                                        in_=sc[:], func=AF.Exp)
                                if stages >= 2:
                                    # Z add-tree (fp16 SBUF, DVE 2x mode)
                                    t8 = trp.tile([128, 8, 256], F16, tag="t8")
                                    if wmode == "gps_t8":
                                        nc.gpsimd.tensor_add(
                                            t8[:], pt[:, 0:8, :], pt[:, 8:16, :])
                                    else:
                                        nc.vector.tensor_add(
                                            t8[:], pt[:, 0:8, :], pt[:, 8:16, :])
                                    t4 = trp.tile([128, 4, 256], F16, tag="t4")
                                    nc.vector.tensor_add(
                                        t4[:], t8[:, 0:4, :], t8[:, 4:8, :])
                                    t2 = trp.tile([128, 2, 256], F16, tag="t2")
                                    nc.vector.tensor_add(
                                        t2[:], t4[:, 0:2, :], t4[:, 2:4, :])
                                    z = trp.tile([128, 256], F32, tag="z")
                                    nc.vector.tensor_add(
                                        z[:], t2[:, 0, :], t2[:, 1, :])
                                    r = trp.tile([128, 256], F32, tag="r")
                                    nc.vector.reciprocal_approx_fast(
                                        out=r[:], in_=z[:])
                                    rm = trp.tile([128, 256], F16, tag="rm")
                                    nc.vector.tensor_mul(
                                        rm[:], r[:], mks[poff + jb][:])
                                if stages >= 3:
                                    # w = P * rm
                                    w = wbuf.tile([128, 8, 256], F16, tag="w")
                                    w2 = w2buf.tile([128, 8, 256], F16,
                                                    tag="w2")
                                    rb8 = rm[:].rearrange(
                                        "p (a f) -> p a f", a=1) \
                                        .to_broadcast([128, 8, 256])
                                    rb4 = rm[:].rearrange(
                                        "p (a f) -> p a f", a=1) \
                                        .to_broadcast([128, 4, 256])
                                    nc.vector.tensor_mul(
                                        w[:], pt[:, 0:8, :], rb8)
                                    if wmode == "mixed":
                                        nc.gpsimd.tensor_mul(
                                            w2[:, 0:4, :], pt[:, 8:12, :], rb4)
                                        nc.gpsimd.tensor_mul(
                                            w2[:, 4:8, :], pt[:, 12:16, :], rb4)
                                    else:
                                        nc.vector.tensor_mul(
                                            w2[:], pt[:, 8:16, :], rb8)
                                    pend.append((p, w, w2))
                            # ---------- phase 2: dense per-super-block chains
                            # (4 consecutive matmuls per (pr, sub) region so
                            # each accumulation group completes before the
                            # next start=True clears the bank's bits)
                            if stages >= 4 and len(pend) == 4:
                                s0 = pend[0][0]
                                ot = otp.tile([128, 8, 256], F32, tag="ot")
                                for pr in range(8):
                                    for sub in range(2):
                                        h = 2 * pr + sub
                                        po = sub * 64
                                        g_, s_ = _HEAD_SLOT[h]
                                        slot = 4 * g_ + s_
                                        for half in range(4):
                                            up, w, w2 = pend[half]
                                            wsrc = w if slot < 8 else w2
                                            wslot = slot if slot < 8 else slot - 8
                                            nc.tensor.matmul(
                                                ot[po:po + 64, pr, :],
                                                vt[:, up, h * 64:(h + 1) * 64],
                                                wsrc[:, wslot, :],
                                                start=(half == 0),
                                                stop=(half == 3),
                                                tile_position=(0, po),
                                            )
                                pend = []
                                base = cks[ci] if s0 == 0 else outT
                                for pr in range(8):
                                    nc.vector.tensor_add(
                                        outT[:, pr, :], ot[:, pr, :],
                                        base[:, pr, :])

                # ---- output projection: nested inside scp's scope so the
                # yacc PSUM tiles land on the freed ot banks, keeping the
                # next repetition's score tiles independent of the tail.
                if stages >= 5:
                    with tc.tile_pool(name="yps", bufs=2, space="PSUM") as yps, \
                         tc.tile_pool(name="ysb", bufs=2) as ysbp:
                        for ci in range(2):
                            for ib in range(2):
                                for nt in range(2):
                                    acc = yps.tile([128, 512], F32, tag="yacc")
                                    for dc in range(8):
                                        nc.tensor.matmul(
                                            acc[:],
                                            outTs[ci][:, dc,
                                                      ib * 128:(ib + 1) * 128],
                                            wot[:, dc,
                                                nt * 512:(nt + 1) * 512],
                                            start=(dc == 0), stop=False,
                                        )
                                    nc.tensor.matmul(
                                        acc[:], ones1[:],
                                        bo_sb[:, nt * 512:(nt + 1) * 512],
                                        start=False, stop=True,
                                    )
                                    yt = ysbp.tile([128, 512], F32, tag="yt")
                                    nc.vector.tensor_copy(yt[:], acc[:])
                                    nc.sync.dma_start(
                                        out=y_o[(ci * 2 + ib) * 128:
                                                (ci * 2 + ib + 1) * 128,
                                                nt * 512:(nt + 1) * 512],
                                        in_=yt[:])
    nc.compile()
    return nc


# ------------------------------------------------------------------- driver
def _masks_for(q):
    """[24, 128, 256] keep-masks for quarter q (padding positions -> 0)."""
    i = np.arange(128)
    tri = (i[:, None] <= i[None, :]).astype(np.float32)   # [j, i], keep j<=i
    ones = np.ones((128, 128), np.float32)
    zeros = np.zeros((128, 128), np.float32)
    out = np.zeros((NPOS, 128, 256), np.float32)
    for ci, (b0, b1) in enumerate(_chunk_blocks(q)):
        cnt_real = b1 + 1                  # real causal j-blocks
        poff = 0 if ci == 0 else CNT[0]
        for p in range(CNT[ci]):
            if p >= cnt_real:
                continue                   # padding: stays zero
            left = tri if p == b0 else (ones if p < b0 else zeros)
            right = tri if p == b1 else (ones if p < b1 else zeros)
            out[poff + p] = np.concatenate([left, right], axis=1)
    return out.astype(np.float16)


def kernel(x, Wq, bq, Wk, bk, Wv, bv, Wo, bo):
    x = np.asarray(x, dtype=np.float32)
    Wq, bq = np.asarray(Wq, np.float32), np.asarray(bq, np.float32)
    Wk, bk = np.asarray(Wk, np.float32), np.asarray(bk, np.float32)
    Wv, bv = np.asarray(Wv, np.float32), np.asarray(bv, np.float32)
    Wo, bo = np.asarray(Wo, np.float32), np.asarray(bo, np.float32)

    if "a" not in _cache:
        _cache["a"] = _build_a()
    if "b" not in _cache:
        _cache["b"] = _build_b()

    def part8(a):  # [1024, N] -> [128, 8, N] partition-major contiguous
        return np.ascontiguousarray(a.reshape(8, 128, -1).transpose(1, 0, 2))

    x_flat = x.reshape(B * T, D)
    wq16, wk16, wv16 = (part8(w.astype(np.float16)) for w in (Wq, Wk, Wv))
    bqT = np.ascontiguousarray(bq.reshape(8, 128).T).astype(np.float32)
    bkT = np.ascontiguousarray((bk / 32.0).reshape(8, 128).T).astype(np.float32)
    bv_row = bv.astype(np.float16)[None, :]
    in_maps_a = []
    for c in range(NC):
        xTs = part8(np.ascontiguousarray(x_flat[c * 512:(c + 1) * 512].T).astype(np.float16))
        in_maps_a.append(dict(xT=xTs, wq=wq16, wk=wk16, wv=wv16,
                              bqT=bqT, bkT=bkT, bv_row=bv_row))
    res_a = run_bass_kernel_spmd(_cache["a"], in_maps_a, core_ids=list(range(NC)))

    def unpart(a):  # [128, C, N] -> [128*C, N]
        return a.transpose(1, 0, 2).reshape(-1, a.shape[2])

    qT_full = [np.concatenate([unpart(res_a.results[b_ * 4 + i]["qT_o"])
                               for i in range(4)], axis=1) for b_ in range(B)]
    kT_full = [np.concatenate([unpart(res_a.results[b_ * 4 + i]["kT_o"])
                               for i in range(4)], axis=1) for b_ in range(B)]
    v_full = [np.concatenate([unpart(res_a.results[b_ * 4 + i]["v_o"])
                              for i in range(4)], axis=0) for b_ in range(B)]

    bo_row = bo.astype(np.float16)[None, :]
    wo16 = part8(Wo.astype(np.float16))
    masks_q = [_masks_for(q) for q in range(4)]

    in_maps_b = []
    for c in range(NC):
        b_, qq = c // 4, c % 4
        (a0, _a1), (b0, _b1) = _chunk_blocks(qq)
        qT_core = np.ascontiguousarray(np.concatenate(
            [qT_full[b_][:, a0 * 128:a0 * 128 + 256],
             qT_full[b_][:, b0 * 128:b0 * 128 + 256]], axis=1))
        vf32 = v_full[b_].astype(np.float32)
        suffix = (vf32.sum(0)[None, :] - np.cumsum(vf32, axis=0)) / 16.0  # [T, D]
        corr = np.zeros((2, 8, 128, 256), np.float32)
        for ci, cblk in enumerate((a0, b0)):
            blk = suffix[cblk * 128: cblk * 128 + 256]          # [256 i, 1024 d]
            corr[ci] = blk.T.reshape(8, 128, 256)
        corr2 = np.ascontiguousarray(corr.transpose(0, 2, 1, 3))  # [2,128,8,256]
        v16 = np.ascontiguousarray(
            v_full[b_].reshape(16, 128, D).transpose(1, 0, 2))
        in_maps_b.append(dict(
            qT=part8(qT_core), kT=part8(np.ascontiguousarray(kT_full[b_])),
            v=v16, wo=wo16, bo_row=bo_row,
            masks=masks_q[qq], corr=corr2.astype(np.float16)))

    res_b = run_bass_kernel_spmd(_cache["b"], in_maps_b, core_ids=list(range(NC)))

    y = np.zeros((B, T, D), np.float32)
    for c in range(NC):
        b_, qq = c // 4, c % 4
        (a0, _), (b0, _) = _chunk_blocks(qq)
        yc = res_b.results[c]["y"]
        y[b_, a0 * 128:a0 * 128 + 256] = yc[:256]
        y[b_, b0 * 128:b0 * 128 + 256] = yc[256:]
    return y
